# revision 6
# baseline (speedup 1.0000x reference)
"""Trainium2 Bass kernel: batched causal attention (B=4, S=4096, E=256, f32).

Collective-free sharding: 2 cores per batch element. Each core holds the FULL
K/V inputs for its batch (duplicated across the pair) and a causally-balanced,
interleaved half of the Q rows, so no cross-core reduction is needed.

Per core: 4 slots of 512 q rows. Slot m holds 4 q-tiles (128 rows each) with
causal extents {8m+8-h-2i} k-tiles (h = core parity, data-only difference),
sorted descending, so slot m needs k-tiles 0..8m+7. The last 8 k-tiles of each
slot run at shrinking free widths w(d) = 128*[4,4,3,3,2,2,1,1][d], which
computes the causal triangle almost exactly (68 tile-units vs 72 for a
ReduceScatter pair split) while keeping one SPMD instruction stream: the
per-parity difference lives entirely in the mask *data* (ones/tri/zeros).

The V projection is folded through the attention: instead of P @ (Z Wv^T),
the PE accumulates U = Z^T-contracted P (stationary = natural Z k-tiles) and
each slot finishes with O^T = Wv U (4 matmuls), so the per-k-tile V
projection disappears. Scores are computed transposed (S^T = K^T . Q^T,
contract e'), exp -> P^T feeds the U accumulation directly (contract k).
Rowsums via ones-stationary matmul; all 128 rowsum PSUM partitions are
identical so normalization is a DVE reciprocal+mul along the free axis,
then +bv via DVE tensor_scalar_add (per-partition scalar). bk dropped
(softmax shift-invariant), bv folded at the end (attn rows sum to 1).

No PE transposes anywhere: the host passes X^T, Z^T, Z, Wq^T/Wk^T/Wv^T
(bf16) and the kernel returns O^T (bf16) which the host transposes back.
Bulk inputs ride gpsimd (SWDGE) DMAs, which fan out across queues; each
dma_start costs ~0.5us of sequencer time, so triggers are ordered by when
their data gates compute. K/Q projection work for later slots is woven
between attention pairs so the PE queue never stalls on DVE copies.
"""

from collections import deque

import numpy as np

B = 4
S = 4096
E = 256
NSLOT = 4
F = 512

# width schedule for the last 8 k-tiles of each slot (in 128-col units)
A_SCHED = (4, 4, 3, 3, 2, 2, 1, 1)

_COMPILED = {}


def _build():
    import concourse.bass as bass
    import concourse.tile as tile
    from concourse import mybir, bacc

    f32 = mybir.dt.float32
    bf16 = mybir.dt.bfloat16
    Exp = mybir.ActivationFunctionType.Exp
    Copy = mybir.ActivationFunctionType.Copy
    Ident = mybir.ActivationFunctionType.Identity

    nc = bacc.Bacc("TRN2", target_bir_lowering=False, debug=False,
                   enable_asserts=True, num_devices=8)

    xt_ext = nc.dram_tensor("xt", [E, S // 2], bf16, kind="ExternalInput")
    zt_ext = nc.dram_tensor("zt", [E, S], bf16, kind="ExternalInput")
    zn_ext = nc.dram_tensor("zn", [128, 32, E], bf16, kind="ExternalInput")
    wts_ext = nc.dram_tensor("wts", [128, 6, E], bf16, kind="ExternalInput")
    bb_ext = nc.dram_tensor("bb", [128, 4], f32, kind="ExternalInput")  # bq/16, bv
    masks_ext = nc.dram_tensor("masks", [128, 9, 128], bf16, kind="ExternalInput")
    outt_ext = nc.dram_tensor("outt", [E, S // 2], bf16, kind="ExternalOutput")

    with tile.TileContext(nc) as tc:
        with tc.tile_pool(name="singles", bufs=1) as singles:
            # ---- bulk inputs: gpsimd (SWDGE) spreads one DMA over ~4 queues;
            # sync/scalar HWDGE each own a single queue. Triggers cost ~0.5us
            # of sequencer time apiece, so they are ordered by data need.
            wT_all = singles.tile([128, 6, E], bf16, tag="wT_all")
            zT_sb = singles.tile([128, 2, S], bf16, tag="zT_sb")
            xT_sb = singles.tile([128, 2, S // 2], bf16, tag="xT_sb")
            zn_sb = singles.tile([128, 32, E], bf16, tag="zn_sb")
            maskt = singles.tile([128, 9, 128], bf16)
            bb = singles.tile([128, 4], f32)

            # Critical first pieces ride sync/scalar HWDGE (fast ~0.6us
            # triggers, one dedicated queue each), ordered by when they gate
            # compute: k weights -> q weights -> first z^T/x^T columns ->
            # x^T tails -> v weights (only needed by the first post phase).
            # Everything bulky rides gpsimd SWDGE (slow trigger, fans out
            # across ~4 queues). wts layout: [q0,q1,k0,k1,v0,v1].
            nc.sync.dma_start(out=wT_all[:, 2, :], in_=wts_ext[:, 2, :])
            nc.scalar.dma_start(out=wT_all[:, 3, :], in_=wts_ext[:, 3, :])
            nc.sync.dma_start(out=wT_all[:, 0, :], in_=wts_ext[:, 0, :])
            nc.scalar.dma_start(out=wT_all[:, 1, :], in_=wts_ext[:, 1, :])
            for et in range(2):
                nc.gpsimd.dma_start(out=zT_sb[:, et, 0:1024],
                                    in_=zt_ext[128 * et:128 * (et + 1), 0:1024])
            nc.sync.dma_start(out=xT_sb[:, 0, 0:512], in_=xt_ext[0:128, 0:512])
            nc.scalar.dma_start(out=xT_sb[:, 1, 0:512], in_=xt_ext[128:256, 0:512])
            nc.sync.dma_start(out=bb[:], in_=bb_ext[:])
            nc.sync.dma_start(out=wT_all[:, 4, :], in_=wts_ext[:, 4, :])
            nc.scalar.dma_start(out=wT_all[:, 5, :], in_=wts_ext[:, 5, :])
            nc.gpsimd.dma_start(out=zn_sb[:, 0:4, :], in_=zn_ext[:, 0:4, :])
            # masks[0..7] = per-d mask tiles, masks[8] = ones (rowsum stationary)
            nc.gpsimd.dma_start(out=maskt[:], in_=masks_ext[:])
            ones_r = maskt[:, 8, :]
            nc.gpsimd.dma_start(out=xT_sb[:, 0, 512:2048], in_=xt_ext[0:128, 512:2048])
            nc.gpsimd.dma_start(out=xT_sb[:, 1, 512:2048], in_=xt_ext[128:256, 512:2048])
            for et in range(2):
                nc.gpsimd.dma_start(out=zT_sb[:, et, 1024:2048],
                                    in_=zt_ext[128 * et:128 * (et + 1), 1024:2048])
            nc.gpsimd.dma_start(out=zn_sb[:, 4:16, :], in_=zn_ext[:, 4:16, :])
            nc.gpsimd.dma_start(out=zn_sb[:, 16:32, :], in_=zn_ext[:, 16:32, :])
            for et in range(2):
                nc.gpsimd.dma_start(out=zT_sb[:, et, 2048:4096],
                                    in_=zt_ext[128 * et:128 * (et + 1), 2048:4096])

            def wT(wname, et):
                widx = {"q": 0, "k": 2, "v": 4}[wname] + et
                return wT_all[:, widx, :]

            # ---- persistent projection outputs ----------------------------
            kT = singles.tile([128, 2, S], bf16, tag="kT")
            qT = singles.tile([128, 2, S // 2], bf16, tag="qT")

            with tc.tile_pool(name="pT", bufs=6) as pTp, \
                 tc.tile_pool(name="post", bufs=4) as post, \
                 tc.tile_pool(name="ps_a", bufs=3, space="PSUM") as psA, \
                 tc.tile_pool(name="ps_o", bufs=2, space="PSUM") as psO, \
                 tc.tile_pool(name="ps_r", bufs=1, space="PSUM") as psR:

                # PE warm-up: dummy matmuls during the input-DMA wait pull the
                # HAM clock gate to 8/8 before real work arrives
                warm = singles.tile([128, F], bf16, name="warm")
                nc.vector.memset(warm[:], 0)
                for _ in range(6):
                    psw = psA.tile([128, F], f32, tag="psA", name="psw")
                    nc.tensor.matmul(psw[:], warm[:, 0:128], warm[:],
                                     start=True, stop=True)

                def k_group(sc, ft):
                    psk = psA.tile([128, F], f32, tag="psA", name="psk")
                    for et in range(2):
                        nc.tensor.matmul(psk[:], wT("k", et)[:, 128 * ft:128 * (ft + 1)],
                                         zT_sb[:, et, 512 * sc:512 * (sc + 1)],
                                         start=(et == 0), stop=(et == 1))
                    # split PSUM evacuation across scalar/DVE so neither backs
                    # up the psA rotation during weave windows
                    if ft == 0:
                        nc.scalar.activation(out=kT[:, ft, 512 * sc:512 * (sc + 1)],
                                             in_=psk[:], func=Copy)
                    else:
                        nc.vector.tensor_copy(out=kT[:, ft, 512 * sc:512 * (sc + 1)],
                                              in_=psk[:])

                def q_group(m, ft):
                    psq = psA.tile([128, F], f32, tag="psA", name="psq")
                    for et in range(2):
                        nc.tensor.matmul(psq[:], wT("q", et)[:, 128 * ft:128 * (ft + 1)],
                                         xT_sb[:, et, 512 * m:512 * (m + 1)],
                                         start=(et == 0), stop=(et == 1))
                    # (psq/16 + bq/16) on DVE: keeps the scalar queue free for
                    # the exp chain during weave windows
                    nc.vector.tensor_scalar(
                        out=qT[:, ft, 512 * m:512 * (m + 1)], in0=psq[:],
                        scalar1=1.0 / 16.0, scalar2=bb[:, ft:ft + 1],
                        op0=mybir.AluOpType.mult, op1=mybir.AluOpType.add)

                def proj_steps():
                    # drain order: z0, q0, z1, q1, z2, z3, q2, z4, z5, q3, z6, z7
                    order = [("z", 0), ("q", 0), ("z", 1), ("q", 1), ("z", 2),
                             ("z", 3), ("q", 2), ("z", 4), ("z", 5), ("q", 3),
                             ("z", 6), ("z", 7)]
                    steps = []
                    for kind, i in order:
                        for ft in range(2):
                            if kind == "z":
                                steps.append((("z", i), lambda sc=i, ft=ft: k_group(sc, ft)))
                            else:
                                steps.append((("q", i), lambda m=i, ft=ft: q_group(m, ft)))
                    return deque(steps)

                steps = proj_steps()

                def drain(m, p):
                    while steps:
                        kind, i = steps[0][0]
                        if (kind == "z" and i <= p // 2) or (kind == "q" and i <= m):
                            steps.popleft()[1]()
                        else:
                            break

                slot_state = {}

                def attn_slot(m, post_hook=None):
                    nkt = 8 * m + 8
                    npair = nkt // 2
                    psu = psO.tile([128, 2 * F], f32, tag="psu", name="psu")
                    psr = psR.tile([128, F], f32, tag="psr", name="psr")
                    slot_state[m] = (psu, psr)
                    for pair in range(npair):
                        drain(m, pair)
                        if pair == 1 and post_hook is not None:
                            post_hook()
                        dp = pair - 4 * m
                        W = F if dp < 0 else 128 * A_SCHED[2 * dp]
                        pT = pTp.tile([128, 2, F], bf16, tag="pT", name="pT")
                        for j in range(2):
                            ll = 2 * pair + j
                            d = ll - 8 * m
                            pss = psA.tile([128, F], f32, tag="psA", name="pss")
                            for et in range(2):
                                nc.tensor.matmul(pss[:, :W], kT[:, et, 128 * ll:128 * (ll + 1)],
                                                 qT[:, et, 512 * m:512 * m + W],
                                                 start=(et == 0), stop=(et == 1))
                            nc.scalar.activation(out=pT[:, j, :W], in_=pss[:, :W], func=Exp)
                            if d >= 0:
                                o = 128 * (3 - d // 2)
                                nc.vector.tensor_mul(pT[:, j, o:o + 128],
                                                     pT[:, j, o:o + 128],
                                                     maskt[:, d, :])
                        first = (pair == 0)
                        last = (pair == npair - 1)
                        for j in range(2):
                            ll = 2 * pair + j
                            for ez in range(2):
                                nc.tensor.matmul(psu[:, F * ez:F * ez + W],
                                                 zn_sb[:, ll, 128 * ez:128 * (ez + 1)],
                                                 pT[:, j, :W],
                                                 start=(first and j == 0),
                                                 stop=(last and j == 1),
                                                 skip_group_check=True)
                            nc.tensor.matmul(psr[:, :W], ones_r, pT[:, j, :W],
                                             start=(first and j == 0),
                                             stop=(last and j == 1),
                                             skip_group_check=True)
                        for _ in range(3):
                            if steps:
                                steps.popleft()[1]()

                def post_a(m):
                    # evacuate U and the rowsum right at slot end (cheap, off-PE)
                    psu, psr = slot_state.pop(m)
                    u_sb = post.tile([128, 2, F], bf16, tag="u_sb", name="u_sb")
                    nc.scalar.activation(out=u_sb[:, 0, :], in_=psu[:, 0:F], func=Copy)
                    nc.vector.tensor_copy(out=u_sb[:, 1, :], in_=psu[:, F:2 * F])
                    recip = post.tile([128, F], f32, tag="recip", name="recip")
                    nc.vector.reciprocal_approx_fast(out=recip[:], in_=psr[:])
                    slot_state[m, "post"] = (u_sb, recip)

                def post_b(m):
                    # O^T = Wv U, normalize, +bv, write out — deferred past the
                    # next slot's first pair so the PE never waits on u_sb
                    u_sb, recip = slot_state.pop((m, "post"))
                    for ft in range(2):
                        psof = psA.tile([128, F], f32, tag="psA", name="psof")
                        for ez in range(2):
                            nc.tensor.matmul(psof[:], wT("v", ez)[:, 128 * ft:128 * (ft + 1)],
                                             u_sb[:, ez, :], start=(ez == 0), stop=(ez == 1))
                        po_n = post.tile([128, F], bf16, tag="po_n", name="po_n")
                        nc.vector.tensor_mul(po_n[:], psof[:], recip[:])
                        po_f = post.tile([128, F], bf16, tag="po_f", name="po_f")
                        nc.vector.tensor_scalar_add(po_f[:], po_n[:], bb[:, 2 + ft:3 + ft])
                        if m == NSLOT - 1:
                            nc.sync.dma_start(
                                out=outt_ext[128 * ft:128 * (ft + 1),
                                             512 * m:512 * m + 256],
                                in_=po_f[:, 0:256])
                            nc.scalar.dma_start(
                                out=outt_ext[128 * ft:128 * (ft + 1),
                                             512 * m + 256:512 * (m + 1)],
                                in_=po_f[:, 256:512])
                        else:
                            eng = nc.sync if ft == 0 else nc.scalar
                            eng.dma_start(
                                out=outt_ext[128 * ft:128 * (ft + 1), 512 * m:512 * (m + 1)],
                                in_=po_f[:])

                pending = None
                for m in range(NSLOT):
                    hook = (lambda mm=pending: post_b(mm)) if pending is not None else None
                    attn_slot(m, post_hook=hook)
                    post_a(m)
                    pending = m
                while steps:
                    steps.popleft()[1]()
                post_b(pending)

    import os
    if not os.environ.get("BASS_SKIP_COMPILE"):
        nc.compile()
    return nc


def _get_nc():
    if "nc" not in _COMPILED:
        _COMPILED["nc"] = _build()
    return _COMPILED["nc"]


def _q_tiles(h):
    return [8 * m + 7 - h - 2 * i for m in range(NSLOT) for i in range(4)]


def kernel(X, Z, mask, Wq, bq, Wk, bk, Wv, bv):
    import ml_dtypes
    bfdt = ml_dtypes.bfloat16

    X = np.asarray(X, dtype=np.float32)
    Z = np.asarray(Z, dtype=np.float32)
    mask_np = np.asarray(mask)

    causal = bool(np.array_equal(
        mask_np != 0, np.tril(np.ones((S, S), dtype=bool))))
    if not causal:
        return _numpy_ref(X, Z, mask_np, Wq, bq, Wk, bk, Wv, bv)

    from concourse.bass_utils import run_bass_kernel_spmd

    nc = _get_nc()

    wts = np.empty((6, 128, E), dtype=bfdt)
    for widx, W_ in enumerate((Wq, Wk, Wv)):
        wt = np.ascontiguousarray(np.asarray(W_, dtype=np.float32).T).astype(bfdt)
        wts[2 * widx] = wt[:128]
        wts[2 * widx + 1] = wt[128:]
    wts = np.ascontiguousarray(wts.transpose(1, 0, 2))  # [128, 6, E]

    bqs = np.asarray(bq, dtype=np.float32) / np.float32(16.0)
    bvf = np.asarray(bv, dtype=np.float32)
    bb = np.ascontiguousarray(
        np.stack([bqs[:128], bqs[128:], bvf[:128], bvf[128:]]).T)  # [128, 4]

    tri = np.triu(np.ones((128, 128), dtype=np.float32))  # keep iff qcol >= krow
    zeros = np.zeros((128, 128), dtype=np.float32)
    ones_f = np.ones((128, 128), dtype=np.float32)
    masks_h = []
    for h in range(2):
        tiles = []
        for d in range(8):
            if h == 0:
                tiles.append(tri if d % 2 == 1 else ones_f)
            else:
                tiles.append(tri if d % 2 == 0 else zeros)
        tiles.append(ones_f)  # slot 8 = rowsum stationary
        masks_h.append(np.ascontiguousarray(
            np.stack(tiles).transpose(1, 0, 2)).astype(bfdt))  # [128, 9, 128]

    zt_b = [np.ascontiguousarray(Z[b].T).astype(bfdt) for b in range(B)]
    zn_b = [np.ascontiguousarray(
        Z[b].reshape(32, 128, E).transpose(1, 0, 2)).astype(bfdt)
        for b in range(B)]  # [128, 32, E]

    in_maps = []
    for c in range(8):
        b, h = c // 2, c % 2
        tiles = _q_tiles(h)
        xb = X[b].reshape(32, 128, E)[tiles].reshape(S // 2, E)
        in_maps.append({
            "xt": np.ascontiguousarray(xb.T).astype(bfdt),
            "zt": zt_b[b],
            "zn": zn_b[b],
            "wts": wts,
            "bb": bb,
            "masks": masks_h[h],
        })

    res = run_bass_kernel_spmd(nc, in_maps, core_ids=list(range(8)))

    out = np.empty((B, S, E), dtype=np.float32)
    for c in range(8):
        b, h = c // 2, c % 2
        tiles = _q_tiles(h)
        out[b].reshape(32, 128, E)[tiles] = np.ascontiguousarray(
            res.results[c]["outt"].astype(np.float32).T).reshape(16, 128, E)
    return out


def _numpy_ref(X, Z, mask, Wq, bq, Wk, bk, Wv, bv):
    q = np.einsum("bse,fe->bsf", X, Wq) + bq
    k = np.einsum("bse,fe->bsf", Z, Wk) + bk
    v = np.einsum("bse,fe->bsf", Z, Wv) + bv
    s = np.einsum("bqe,bke->bqk", q, k) / np.sqrt(np.float32(X.shape[-1]))
    s = np.where(mask == 0, -np.inf, s)
    s = s - s.max(axis=-1, keepdims=True)
    p = np.exp(s)
    p /= p.sum(axis=-1, keepdims=True)
    return np.einsum("bqk,bke->bqe", p, v).astype(np.float32)
